# revision 13
# baseline (speedup 1.0000x reference)
"""CondConv2D Trainium2 kernel (v3).

Problem (hardcoded): B=16, C_in=64, H=W=256, E=4, C_out=64, 3x3 conv,
stride=1, dilation=1, padding=1.

Sharding: data-parallel over batch. 8 cores x 2 images each. Expert weights
and routing fc params replicated (host pre-transposed for layout only).

Per-core kernel:
  - Each image resident in SBUF as ONE flat tile [128, 130, 256] bf16:
    partitions 0-63 hold channel c's TOP rows (slot m = global row m-1,
    incl. zero pad row -1), partitions 64-127 the BOTTOM rows
    (slot m = global row 127+m, incl. zero pad row 256). Loaded in 5 row
    chunks (SWDGE f32->bf16 cast DMAs). Flat layout => no row-pair ever
    straddles a tile boundary.
  - Routing pool image 0: per-chunk DVE reduce (DVE idle during the head);
    ones-matmul partition combine on the (idle) PE; wmix on DVE.
  - Routing pool image 1 runs during image 0's conv, off the drain engines:
    ACT activation(Copy, accum_out) per chunk (in place), combine via
    gpsimd.partition_all_reduce, sigmoid on ACT, wmix as ACT per-expert
    scalar muls + DVE adds. Emission is interleaved into image 0's drain
    loop in dataflow order (Tile derives dependencies from emission order).
  - Conv: 32 steps/image; step i computes 4 output row-pairs via 4-quadrant
    PE packing: top pair i (0,0)->psX[0:64], top pair 32+i (0,64)->
    psX[64:128], bottom pair i (64,0)->psY[0:64], bottom pair 32+i
    (64,64)->psY[64:128]. Every tap is one matmul (N=512 center column,
    N=510 shifted columns). Tap-outer over groups of NB=3 steps keeps
    same-stationary matmuls back-to-back.
  - psX partitions hold (top q0 | top q1) rows at the same slot, psY
    (bottom q0 | bottom q1) => drains are partition-uniform copies into
    stage tiles [128, 12, 256] f32. Image 0 drains all on DVE (ACT is busy
    pooling image 1); image 1 drains split ACT/DVE.
  - Output: per 6-step block, 4 HWDGE DMAs with 12KB-contiguous
    per-partition descriptors; 64-partition halves split across BOTH HWDGE
    rings (sync + scalar) so even/odd SDMA engine sets run concurrently.
"""
import sys

if "/opt/trn_rl_repo" not in sys.path:
    sys.path.insert(0, "/opt/trn_rl_repo")

import numpy as np

import concourse.bacc as bacc
import concourse.mybir as mybir
import concourse.tile as tile
from concourse import bass_isa
from concourse.bass_utils import run_bass_kernel_spmd

F32 = mybir.dt.float32
BF16 = mybir.dt.bfloat16
AF = mybir.ActivationFunctionType
ALU = mybir.AluOpType
AX = mybir.AxisListType

N_CORES = 8
IMGS_PER_CORE = 2
C_IN = 64
C_OUT = 64
H = 256
W = 256
E = 4
NTAP = 9
SLOTS = 130           # 129 data rows + 1 pad row per half
CHUNK_BOUNDS = [0, 28, 56, 84, 112, 130]   # load/pool chunk slot ranges
NCHUNK = len(CHUNK_BOUNDS) - 1
NB = 3                # conv steps per tap-outer group (2 psum banks/step)
SPB = 6               # conv steps per output stage block
STAGE_ROWS = 2 * SPB  # output rows per quarter per stage block
N_STEPS = 32          # steps per image (each step = 4 output row-pairs)
# center-column taps first (start=True must cover the full psum bank)
TAP_ORDER = [4, 1, 7, 3, 5, 0, 2, 6, 8]
NPX = 2  # legacy knob for test.py compatibility


def _emit_loads(nc, x, xt, i):
    """Chunked SWDGE cast loads for image i into flat tile xt."""
    for k in range(NCHUNK):
        s, e = CHUNK_BOUNDS[k], CHUNK_BOUNDS[k + 1]   # slot range
        ts = max(s, 1)                                # top slot 0 is pad
        nc.gpsimd.dma_start(xt[0:64, ts:e, :], x[i, :, ts - 1:e - 1, :])
        be = min(e, SLOTS - 1)                        # bottom slot 129 is pad
        nc.gpsimd.dma_start(xt[64:128, s:be, :], x[i, :, 127 + s:127 + be, :])


def build_nc(npx=2):
    nc = bacc.Bacc("TRN2", target_bir_lowering=False, debug=False,
                   num_devices=N_CORES)
    x = nc.dram_tensor("x", [IMGS_PER_CORE, C_IN, H, W], F32,
                       kind="ExternalInput")
    wt = nc.dram_tensor("wt", [128, E * NTAP * C_OUT], F32,
                        kind="ExternalInput")
    fcw = nc.dram_tensor("fcw", [128, E], F32, kind="ExternalInput")
    fcb = nc.dram_tensor("fcb", [128, E], F32, kind="ExternalInput")
    ones = nc.dram_tensor("ones", [128, 128], F32, kind="ExternalInput")
    y = nc.dram_tensor("y", [IMGS_PER_CORE, C_OUT, H, W], F32,
                       kind="ExternalOutput")
    S = NTAP * C_OUT

    with tile.TileContext(nc) as tc:
        with (
            tc.tile_pool(name="consts", bufs=1) as consts,
            tc.tile_pool(name="img", bufs=IMGS_PER_CORE) as img_pool,
            tc.tile_pool(name="small", bufs=1) as small,
            tc.tile_pool(name="mix", bufs=1) as mix_pool,
            tc.tile_pool(name="stage", bufs=2) as stage_pool,
            tc.tile_pool(name="psum", bufs=7, space="PSUM") as psum_pool,
        ):
            wtt = consts.tile([128, E * S], BF16)
            fcwt = consts.tile([128, E], F32)
            fcbt = consts.tile([128, E], F32)
            onest = consts.tile([128, 128], F32)
            nc.sync.dma_start(fcwt[:], fcw[:])
            nc.sync.dma_start(fcbt[:], fcb[:])
            nc.sync.dma_start(onest[:], ones[:])

            # ---- phase A: tiles, memsets, loads, image-0 pooling ----
            xt = [None] * IMGS_PER_CORE
            partial = [None] * IMGS_PER_CORE
            for i in range(IMGS_PER_CORE):
                xt[i] = img_pool.tile([128, SLOTS, W], BF16, name="xt")
                nc.vector.memset(xt[i][0:64, 0:1, :], 0.0)
                nc.vector.memset(xt[i][64:128, SLOTS - 1:SLOTS, :], 0.0)
                partial[i] = small.tile([128, NCHUNK + 1], F32,
                                        name=f"partial{i}")
                nc.vector.memset(partial[i][64:128, NCHUNK:NCHUNK + 1], 0.0)

            _emit_loads(nc, x, xt[0], 0)
            # image 0 pooling on DVE, chunk-paced
            for k in range(NCHUNK):
                s, e = CHUNK_BOUNDS[k], CHUNK_BOUNDS[k + 1]
                # chunk 0 starts at slot 2: skips top slots 0-1 (below) and
                # bottom slots 0-1 (rows 127/128, counted in the top half)
                ps = 2 if k == 0 else s
                nc.vector.reduce_sum(partial[0][:, k:k + 1],
                                     xt[0][:, ps:e, :], axis=AX.XY)
            nc.vector.reduce_sum(partial[0][0:64, NCHUNK:NCHUNK + 1],
                                 xt[0][0:64, 0:2, :], axis=AX.XY)
            # wtt after image 0's chunks (only needed once pooling is done)
            nc.gpsimd.dma_start(wtt[:], wt[:])
            _emit_loads(nc, x, xt[1], 1)

            # ---- phase B: image 0 routing + wmix (DVE + idle PE) ----
            pooled0 = small.tile([128, 1], F32)
            nc.vector.reduce_sum(pooled0[:], partial[0][:], axis=AX.X)
            tmp40 = small.tile([128, E], F32)
            nc.vector.tensor_scalar(tmp40[:], fcwt[:], pooled0[:, 0:1],
                                    1.0 / float(H * W),
                                    op0=ALU.mult, op1=ALU.mult)
            ps4 = psum_pool.tile([128, E], F32, bufs=1)
            nc.tensor.matmul(ps4[:], onest[:], tmp40[:], start=True, stop=True)
            logits0 = small.tile([128, E], F32)
            nc.vector.tensor_tensor(logits0[:], ps4[:], fcbt[:], op=ALU.add)
            rt0 = small.tile([128, E], F32)
            nc.scalar.activation(rt0[:], logits0[:], AF.Sigmoid)
            wmix = [None] * IMGS_PER_CORE
            wmix[0] = mix_pool.tile([128, S], BF16, name="wmix0")
            wtmp0 = mix_pool.tile([128, S], BF16)
            nc.vector.tensor_scalar_mul(wmix[0][:], wtt[:, 0:S], rt0[:, 0:1])
            for e_ in range(1, E):
                nc.vector.tensor_scalar_mul(wtmp0[:],
                                            wtt[:, e_ * S:(e_ + 1) * S],
                                            rt0[:, e_:e_ + 1])
                nc.vector.tensor_tensor(wmix[0][:], wmix[0][:], wtmp0[:],
                                        op=ALU.add)

            # ---- image 1 routing pieces (emitted into image 0's conv) ----
            pooled1 = small.tile([128, 1], F32)
            tmp41 = small.tile([128, E], F32)
            bcast1 = small.tile([128, E], F32)
            logits1 = small.tile([128, E], F32)
            rt1 = small.tile([128, E], F32)
            wmix[1] = mix_pool.tile([128, S], BF16, name="wmix1")
            wtA = mix_pool.tile([128, S], BF16)
            wtB = mix_pool.tile([128, S], BF16)

            def emit_pool1(k):
                # ACT pooling: in-place Copy with free-dim accumulator
                s, e = CHUNK_BOUNDS[k], CHUNK_BOUNDS[k + 1]
                ps = 2 if k == 0 else s
                nc.scalar.activation(xt[1][:, ps:e, :], xt[1][:, ps:e, :],
                                     AF.Copy,
                                     accum_out=partial[1][:, k:k + 1])

            def emit_pool1_final():
                nc.scalar.activation(xt[1][0:64, 0:2, :], xt[1][0:64, 0:2, :],
                                     AF.Copy,
                                     accum_out=partial[1][0:64,
                                                         NCHUNK:NCHUNK + 1])
                nc.scalar.activation(partial[1][:], partial[1][:], AF.Copy,
                                     accum_out=pooled1[:])

            def emit_routing1():
                nc.gpsimd.tensor_scalar(tmp41[:], fcwt[:], pooled1[:, 0:1],
                                        1.0 / float(H * W),
                                        op0=ALU.mult, op1=ALU.mult)
                nc.gpsimd.partition_all_reduce(bcast1[:], tmp41[:],
                                               channels=128,
                                               reduce_op=bass_isa.ReduceOp.add)
                nc.gpsimd.tensor_tensor(logits1[:], bcast1[:], fcbt[:],
                                        op=ALU.add)
                nc.scalar.activation(rt1[:], logits1[:], AF.Sigmoid)
                # wmix1 = sum_e r_e W_e: per-expert muls on ACT...
                nc.scalar.mul(wmix[1][:], wtt[:, 0:S], rt1[:, 0:1])
                nc.scalar.mul(wtA[:], wtt[:, S:2 * S], rt1[:, 1:2])
                nc.scalar.mul(wtB[:], wtt[:, 2 * S:3 * S], rt1[:, 2:3])

            def emit_wmix1_addA():
                nc.vector.tensor_tensor(wmix[1][:], wmix[1][:], wtA[:],
                                        op=ALU.add)

            def emit_wmix1_mul3():
                nc.scalar.mul(wtA[:], wtt[:, 3 * S:4 * S], rt1[:, 3:4])

            def emit_wmix1_addB():
                nc.vector.tensor_tensor(wmix[1][:], wmix[1][:], wtB[:],
                                        op=ALU.add)

            def emit_wmix1_addA2():
                nc.vector.tensor_tensor(wmix[1][:], wmix[1][:], wtA[:],
                                        op=ALU.add)

            # emission schedule inside image 0's drain loop (ACT/DVE/GPS
            # pieces placed so no queue blocks on far-future data)
            events0 = {
                2: [lambda: emit_pool1(1)],
                7: [lambda: emit_pool1(2)],
                12: [lambda: emit_pool1(3)],
                16: [lambda: emit_pool1(4)],
                18: [emit_pool1_final],
                19: [emit_routing1],
                22: [emit_wmix1_addA, emit_wmix1_mul3],
                26: [emit_wmix1_addB],
                28: [emit_wmix1_addA2],
            }
            emit_pool1(0)   # ACT idles until image 1's first chunk lands

            # ---- phase C: conv per image ----
            for i in range(IMGS_PER_CORE):
                xti = xt[i]
                wm = wmix[i]
                stageT = None
                stageB = None
                step = 0
                while step < N_STEPS:
                    nsteps = min(NB, N_STEPS - step)
                    steps = list(range(step, step + nsteps))
                    step += nsteps
                    psX = [psum_pool.tile([128, 2, W], F32, name="ps",
                                          tag="ps") for _ in steps]
                    psY = [psum_pool.tile([128, 2, W], F32, name="ps",
                                          tag="ps") for _ in steps]
                    for tap in TAP_ORDER:
                        kh, kw = divmod(tap, 3)
                        st = tap == TAP_ORDER[0]
                        sp = tap == TAP_ORDER[-1]
                        lT = wm[0:64, tap * 64:(tap + 1) * 64]
                        lB = wm[64:128, tap * 64:(tap + 1) * 64]
                        for j, sidx in enumerate(steps):
                            units = (
                                (psX[j], lT, 2 * sidx + kh, (0, 0),
                                 slice(0, 64), slice(0, 64)),
                                (psX[j], lT, 2 * (32 + sidx) + kh, (0, 64),
                                 slice(64, 128), slice(0, 64)),
                                (psY[j], lB, 2 * sidx + kh, (64, 0),
                                 slice(0, 64), slice(64, 128)),
                                (psY[j], lB, 2 * (32 + sidx) + kh, (64, 64),
                                 slice(64, 128), slice(64, 128)),
                            )
                            for ps, lhsT, m, tp, osl, hs in units:
                                if kw == 1:
                                    rhs = xti[hs, m:m + 2, :]
                                    rhs = rhs.rearrange("p a b -> p (a b)")
                                    out = ps[osl].rearrange("p a b -> p (a b)")
                                    nc.tensor.matmul(out, lhsT, rhs,
                                                     start=st, stop=sp,
                                                     tile_position=tp,
                                                     skip_group_check=True)
                                elif kw == 0:
                                    # out col w <- in col w-1
                                    nc.tensor.matmul(
                                        ps[osl, :, 1:W], lhsT,
                                        xti[hs, m:m + 2, 0:W - 1],
                                        start=st, stop=sp, tile_position=tp,
                                        skip_group_check=True)
                                else:
                                    nc.tensor.matmul(
                                        ps[osl, :, 0:W - 1], lhsT,
                                        xti[hs, m:m + 2, 1:W],
                                        start=st, stop=sp, tile_position=tp,
                                        skip_group_check=True)
                    # drains + stage lifecycle + deferred emissions
                    for j, sidx in enumerate(steps):
                        if sidx % SPB == 0:
                            stageT = stage_pool.tile([128, STAGE_ROWS, W],
                                                     F32)
                            stageB = stage_pool.tile([128, STAGE_ROWS, W],
                                                     F32)
                        r0 = (sidx % SPB) * 2
                        # image 0: both drains on DVE (ACT pools image 1);
                        # image 1: split X->ACT / Y->DVE
                        if i == 1:
                            nc.scalar.copy(stageT[:, r0:r0 + 2, :], psX[j][:])
                        else:
                            nc.vector.tensor_copy(stageT[:, r0:r0 + 2, :],
                                                  psX[j][:])
                        nc.vector.tensor_copy(stageB[:, r0:r0 + 2, :],
                                              psY[j][:])
                        if i == 0 and sidx in events0:
                            for fn in events0[sidx]:
                                fn()
                        if sidx % SPB == SPB - 1 or sidx == N_STEPS - 1:
                            nrows = 2 * (sidx % SPB + 1)
                            b0 = (sidx - sidx % SPB) * 2
                            nc.sync.dma_start(
                                y[i, :, b0:b0 + nrows, :],
                                stageT[0:64, 0:nrows, :])
                            nc.scalar.dma_start(
                                y[i, :, 64 + b0:64 + b0 + nrows, :],
                                stageT[64:128, 0:nrows, :])
                            nc.sync.dma_start(
                                y[i, :, 128 + b0:128 + b0 + nrows, :],
                                stageB[0:64, 0:nrows, :])
                            nc.scalar.dma_start(
                                y[i, :, 192 + b0:192 + b0 + nrows, :],
                                stageB[64:128, 0:nrows, :])
    nc.compile()
    return nc


_NC_CACHE = {}


def _get_nc(npx=2):
    if npx not in _NC_CACHE:
        _NC_CACHE[npx] = build_nc(npx)
    return _NC_CACHE[npx]


def _prep_shared(weight, fc_w, fc_b):
    # [E, O, I, KH, KW] -> [I, E, KH, KW, O] -> [64, E*9*64], dup halves
    wt = np.ascontiguousarray(weight.transpose(2, 0, 3, 4, 1)).reshape(
        C_IN, E * NTAP * C_OUT)
    wt = np.concatenate([wt, wt], axis=0).astype(np.float32)
    fcw = np.concatenate([fc_w.T, fc_w.T], axis=0).astype(np.float32)
    fcb = np.tile(fc_b.reshape(1, E), (128, 1)).astype(np.float32)
    ones = np.ones((128, 128), np.float32)
    return wt, fcw, fcb, ones


def kernel(inputs, weight, fc_w, fc_b, stride=1, dilation=1, padding=1,
           _trace=False, _npx=2):
    assert int(stride) == 1 and int(dilation) == 1 and int(padding) == 1
    inputs = np.asarray(inputs, dtype=np.float32)
    B = inputs.shape[0]
    assert B == N_CORES * IMGS_PER_CORE
    wt, fcw, fcb, ones = _prep_shared(np.asarray(weight), np.asarray(fc_w),
                                      np.asarray(fc_b))
    nc = _get_nc(_npx)
    in_maps = []
    for c in range(N_CORES):
        in_maps.append({
            "x": np.ascontiguousarray(inputs[2 * c:2 * c + 2]),
            "wt": wt, "fcw": fcw, "fcb": fcb, "ones": ones,
        })
    res = run_bass_kernel_spmd(nc, in_maps, core_ids=list(range(N_CORES)),
                               trace=_trace)
    out = np.concatenate([res.results[c]["y"] for c in range(N_CORES)], axis=0)
    if _trace:
        return out, res
    return out


# revision 14
# speedup vs baseline: 1.0272x; 1.0272x over previous
"""CondConv2D Trainium2 kernel (v3).

Problem (hardcoded): B=16, C_in=64, H=W=256, E=4, C_out=64, 3x3 conv,
stride=1, dilation=1, padding=1.

Sharding: data-parallel over batch. 8 cores x 2 images each. Expert weights
and routing fc params replicated (host pre-transposed for layout only).

Per-core kernel:
  - Each image resident in SBUF as ONE flat tile [128, 130, 256] bf16:
    partitions 0-63 hold channel c's TOP rows (slot m = global row m-1,
    incl. zero pad row -1), partitions 64-127 the BOTTOM rows
    (slot m = global row 127+m, incl. zero pad row 256). Loaded in 5 row
    chunks (SWDGE f32->bf16 cast DMAs). Flat layout => no row-pair ever
    straddles a tile boundary.
  - Routing pool image 0: per-chunk DVE reduce (DVE idle during the head);
    ones-matmul partition combine on the (idle) PE; wmix on DVE.
  - Routing pool image 1 runs during image 0's conv, off the drain engines:
    ACT activation(Copy, accum_out) per chunk (in place), combine via
    gpsimd.partition_all_reduce, sigmoid on ACT, wmix as ACT per-expert
    scalar muls + DVE adds. Emission is interleaved into image 0's drain
    loop in dataflow order (Tile derives dependencies from emission order).
  - Conv: 32 steps/image; step i computes 4 output row-pairs via 4-quadrant
    PE packing: top pair i (0,0)->psX[0:64], top pair 32+i (0,64)->
    psX[64:128], bottom pair i (64,0)->psY[0:64], bottom pair 32+i
    (64,64)->psY[64:128]. Every tap is one matmul (N=512 center column,
    N=510 shifted columns). Tap-outer over groups of NB=3 steps keeps
    same-stationary matmuls back-to-back.
  - psX partitions hold (top q0 | top q1) rows at the same slot, psY
    (bottom q0 | bottom q1) => drains are partition-uniform copies into
    stage tiles [128, 12, 256] f32. Image 0 drains all on DVE (ACT is busy
    pooling image 1); image 1 drains split ACT/DVE.
  - Output: per 6-step block, 4 HWDGE DMAs with 12KB-contiguous
    per-partition descriptors; 64-partition halves split across BOTH HWDGE
    rings (sync + scalar) so even/odd SDMA engine sets run concurrently.
"""
import sys

if "/opt/trn_rl_repo" not in sys.path:
    sys.path.insert(0, "/opt/trn_rl_repo")

import numpy as np

import concourse.bacc as bacc
import concourse.mybir as mybir
import concourse.tile as tile
from concourse import bass_isa
from concourse.bass_utils import run_bass_kernel_spmd

F32 = mybir.dt.float32
BF16 = mybir.dt.bfloat16
AF = mybir.ActivationFunctionType
ALU = mybir.AluOpType
AX = mybir.AxisListType

N_CORES = 8
IMGS_PER_CORE = 2
C_IN = 64
C_OUT = 64
H = 256
W = 256
E = 4
NTAP = 9
SLOTS = 130           # 129 data rows + 1 pad row per half
CHUNK_BOUNDS = [0, 28, 56, 84, 112, 130]   # load/pool chunk slot ranges
NCHUNK = len(CHUNK_BOUNDS) - 1
NB = 3                # conv steps per tap-outer group (2 psum banks/step)
SPB = 6               # conv steps per output stage block
STAGE_ROWS = 2 * SPB  # output rows per quarter per stage block
N_STEPS = 32          # steps per image (each step = 4 output row-pairs)
# center-column taps first (start=True must cover the full psum bank)
TAP_ORDER = [4, 1, 7, 3, 5, 0, 2, 6, 8]
NPX = 2  # legacy knob for test.py compatibility


def _emit_loads(nc, x, xt, i):
    """Chunked SWDGE cast loads for image i into flat tile xt."""
    for k in range(NCHUNK):
        s, e = CHUNK_BOUNDS[k], CHUNK_BOUNDS[k + 1]   # slot range
        ts = max(s, 1)                                # top slot 0 is pad
        nc.gpsimd.dma_start(xt[0:64, ts:e, :], x[i, :, ts - 1:e - 1, :])
        be = min(e, SLOTS - 1)                        # bottom slot 129 is pad
        nc.gpsimd.dma_start(xt[64:128, s:be, :], x[i, :, 127 + s:127 + be, :])


def build_nc(npx=2):
    nc = bacc.Bacc("TRN2", target_bir_lowering=False, debug=False,
                   num_devices=N_CORES)
    x = nc.dram_tensor("x", [IMGS_PER_CORE, C_IN, H, W], F32,
                       kind="ExternalInput")
    wt = nc.dram_tensor("wt", [128, E * NTAP * C_OUT], F32,
                        kind="ExternalInput")
    fcw = nc.dram_tensor("fcw", [128, E], F32, kind="ExternalInput")
    fcb = nc.dram_tensor("fcb", [128, E], F32, kind="ExternalInput")
    ones = nc.dram_tensor("ones", [128, 128], F32, kind="ExternalInput")
    y = nc.dram_tensor("y", [IMGS_PER_CORE, C_OUT, H, W], F32,
                       kind="ExternalOutput")
    S = NTAP * C_OUT

    with tile.TileContext(nc) as tc:
        with (
            tc.tile_pool(name="consts", bufs=1) as consts,
            tc.tile_pool(name="img", bufs=IMGS_PER_CORE) as img_pool,
            tc.tile_pool(name="small", bufs=1) as small,
            tc.tile_pool(name="mix", bufs=1) as mix_pool,
            tc.tile_pool(name="stage", bufs=2) as stage_pool,
            tc.tile_pool(name="psum", bufs=7, space="PSUM") as psum_pool,
        ):
            wtt = consts.tile([128, E * S], BF16)
            fcwt = consts.tile([128, E], F32)
            fcbt = consts.tile([128, E], F32)
            onest = consts.tile([128, 128], F32)
            nc.sync.dma_start(fcwt[:], fcw[:])
            nc.sync.dma_start(fcbt[:], fcb[:])
            nc.sync.dma_start(onest[:], ones[:])

            # ---- phase A: tiles, memsets, loads, image-0 pooling ----
            xt = [None] * IMGS_PER_CORE
            partial = [None] * IMGS_PER_CORE
            for i in range(IMGS_PER_CORE):
                xt[i] = img_pool.tile([128, SLOTS, W], BF16, name="xt")
                nc.vector.memset(xt[i][0:64, 0:1, :], 0.0)
                nc.vector.memset(xt[i][64:128, SLOTS - 1:SLOTS, :], 0.0)
                partial[i] = small.tile([128, NCHUNK + 1], F32,
                                        name=f"partial{i}")
                nc.vector.memset(partial[i][64:128, NCHUNK:NCHUNK + 1], 0.0)

            _emit_loads(nc, x, xt[0], 0)
            # image 0 pooling on DVE, chunk-paced
            for k in range(NCHUNK):
                s, e = CHUNK_BOUNDS[k], CHUNK_BOUNDS[k + 1]
                # chunk 0 starts at slot 2: skips top slots 0-1 (below) and
                # bottom slots 0-1 (rows 127/128, counted in the top half)
                ps = 2 if k == 0 else s
                nc.vector.reduce_sum(partial[0][:, k:k + 1],
                                     xt[0][:, ps:e, :], axis=AX.XY)
            nc.vector.reduce_sum(partial[0][0:64, NCHUNK:NCHUNK + 1],
                                 xt[0][0:64, 0:2, :], axis=AX.XY)
            # wtt after image 0's chunks (only needed once pooling is done)
            nc.gpsimd.dma_start(wtt[:], wt[:])
            _emit_loads(nc, x, xt[1], 1)

            # ---- phase B: image 0 routing + wmix (DVE + idle PE) ----
            pooled0 = small.tile([128, 1], F32)
            nc.vector.reduce_sum(pooled0[:], partial[0][:], axis=AX.X)
            tmp40 = small.tile([128, E], F32)
            nc.vector.tensor_scalar(tmp40[:], fcwt[:], pooled0[:, 0:1],
                                    1.0 / float(H * W),
                                    op0=ALU.mult, op1=ALU.mult)
            ps4 = psum_pool.tile([128, E], F32, bufs=1)
            nc.tensor.matmul(ps4[:], onest[:], tmp40[:], start=True, stop=True)
            logits0 = small.tile([128, E], F32)
            nc.vector.tensor_tensor(logits0[:], ps4[:], fcbt[:], op=ALU.add)
            rt0 = small.tile([128, E], F32)
            nc.scalar.activation(rt0[:], logits0[:], AF.Sigmoid)
            wmix = [None] * IMGS_PER_CORE
            wmix[0] = mix_pool.tile([128, S], BF16, name="wmix0")
            wtmp0 = mix_pool.tile([128, S], BF16)
            nc.vector.tensor_scalar_mul(wmix[0][:], wtt[:, 0:S], rt0[:, 0:1])
            for e_ in range(1, E):
                nc.vector.tensor_scalar_mul(wtmp0[:],
                                            wtt[:, e_ * S:(e_ + 1) * S],
                                            rt0[:, e_:e_ + 1])
                nc.vector.tensor_tensor(wmix[0][:], wmix[0][:], wtmp0[:],
                                        op=ALU.add)

            # ---- image 1 routing pieces (emitted into image 0's conv) ----
            pooled1 = small.tile([128, 1], F32)
            tmp41 = small.tile([128, E], F32)
            bcast1 = small.tile([128, E], F32)
            logits1 = small.tile([128, E], F32)
            rt1 = small.tile([128, E], F32)
            wmix[1] = mix_pool.tile([128, S], BF16, name="wmix1")
            wtA = mix_pool.tile([128, S], BF16)
            wtB = mix_pool.tile([128, S], BF16)

            def emit_pool1(k):
                # DVE pooling chunk (Y-drain queue shares DVE; emission
                # points below are matched to chunk-k DMA arrival times)
                s, e = CHUNK_BOUNDS[k], CHUNK_BOUNDS[k + 1]
                ps = 2 if k == 0 else s
                nc.vector.reduce_sum(partial[1][:, k:k + 1],
                                     xt[1][:, ps:e, :], axis=AX.XY)

            def emit_pool1_final():
                nc.vector.reduce_sum(partial[1][0:64, NCHUNK:NCHUNK + 1],
                                     xt[1][0:64, 0:2, :], axis=AX.XY)
                nc.vector.reduce_sum(pooled1[:], partial[1][:], axis=AX.X)

            def emit_routing1():
                nc.gpsimd.tensor_scalar(tmp41[:], fcwt[:], pooled1[:, 0:1],
                                        1.0 / float(H * W),
                                        op0=ALU.mult, op1=ALU.mult)
                nc.gpsimd.partition_all_reduce(bcast1[:], tmp41[:],
                                               channels=128,
                                               reduce_op=bass_isa.ReduceOp.add)
                nc.gpsimd.tensor_tensor(logits1[:], bcast1[:], fcbt[:],
                                        op=ALU.add)
                nc.scalar.activation(rt1[:], logits1[:], AF.Sigmoid)
                # wmix1 = sum_e r_e W_e: per-expert muls on ACT...
                nc.scalar.mul(wmix[1][:], wtt[:, 0:S], rt1[:, 0:1])
                nc.scalar.mul(wtA[:], wtt[:, S:2 * S], rt1[:, 1:2])
                nc.scalar.mul(wtB[:], wtt[:, 2 * S:3 * S], rt1[:, 2:3])

            def emit_wmix1_addA():
                nc.vector.tensor_tensor(wmix[1][:], wmix[1][:], wtA[:],
                                        op=ALU.add)

            def emit_wmix1_mul3():
                nc.scalar.mul(wtA[:], wtt[:, 3 * S:4 * S], rt1[:, 3:4])

            def emit_wmix1_addB():
                nc.vector.tensor_tensor(wmix[1][:], wmix[1][:], wtB[:],
                                        op=ALU.add)

            def emit_wmix1_addA2():
                nc.vector.tensor_tensor(wmix[1][:], wmix[1][:], wtA[:],
                                        op=ALU.add)

            # emission schedule inside image 0's drain loop (ACT/DVE/GPS
            # pieces placed so no queue blocks on far-future data)
            events0 = {
                1: [lambda: emit_pool1(0)],
                5: [lambda: emit_pool1(1)],
                9: [lambda: emit_pool1(2)],
                13: [lambda: emit_pool1(3)],
                18: [lambda: emit_pool1(4)],
                21: [emit_pool1_final],
                22: [emit_routing1],
                24: [emit_wmix1_addA, emit_wmix1_mul3],
                26: [emit_wmix1_addB],
                28: [emit_wmix1_addA2],
            }

            # ---- phase C: conv per image ----
            for i in range(IMGS_PER_CORE):
                xti = xt[i]
                wm = wmix[i]
                stageT = None
                stageB = None
                step = 0
                while step < N_STEPS:
                    nsteps = min(NB, N_STEPS - step)
                    steps = list(range(step, step + nsteps))
                    step += nsteps
                    psX = [psum_pool.tile([128, 2, W], F32, name="ps",
                                          tag="ps") for _ in steps]
                    psY = [psum_pool.tile([128, 2, W], F32, name="ps",
                                          tag="ps") for _ in steps]
                    for tap in TAP_ORDER:
                        kh, kw = divmod(tap, 3)
                        st = tap == TAP_ORDER[0]
                        sp = tap == TAP_ORDER[-1]
                        lT = wm[0:64, tap * 64:(tap + 1) * 64]
                        lB = wm[64:128, tap * 64:(tap + 1) * 64]
                        for j, sidx in enumerate(steps):
                            units = (
                                (psX[j], lT, 2 * sidx + kh, (0, 0),
                                 slice(0, 64), slice(0, 64)),
                                (psX[j], lT, 2 * (32 + sidx) + kh, (0, 64),
                                 slice(64, 128), slice(0, 64)),
                                (psY[j], lB, 2 * sidx + kh, (64, 0),
                                 slice(0, 64), slice(64, 128)),
                                (psY[j], lB, 2 * (32 + sidx) + kh, (64, 64),
                                 slice(64, 128), slice(64, 128)),
                            )
                            for ps, lhsT, m, tp, osl, hs in units:
                                if kw == 1:
                                    rhs = xti[hs, m:m + 2, :]
                                    rhs = rhs.rearrange("p a b -> p (a b)")
                                    out = ps[osl].rearrange("p a b -> p (a b)")
                                    nc.tensor.matmul(out, lhsT, rhs,
                                                     start=st, stop=sp,
                                                     tile_position=tp,
                                                     skip_group_check=True)
                                elif kw == 0:
                                    # out col w <- in col w-1
                                    nc.tensor.matmul(
                                        ps[osl, :, 1:W], lhsT,
                                        xti[hs, m:m + 2, 0:W - 1],
                                        start=st, stop=sp, tile_position=tp,
                                        skip_group_check=True)
                                else:
                                    nc.tensor.matmul(
                                        ps[osl, :, 0:W - 1], lhsT,
                                        xti[hs, m:m + 2, 1:W],
                                        start=st, stop=sp, tile_position=tp,
                                        skip_group_check=True)
                    # drains + stage lifecycle + deferred emissions
                    for j, sidx in enumerate(steps):
                        if sidx % SPB == 0:
                            stageT = stage_pool.tile([128, STAGE_ROWS, W],
                                                     F32)
                            stageB = stage_pool.tile([128, STAGE_ROWS, W],
                                                     F32)
                        r0 = (sidx % SPB) * 2
                        nc.scalar.copy(stageT[:, r0:r0 + 2, :], psX[j][:])
                        nc.vector.tensor_copy(stageB[:, r0:r0 + 2, :],
                                              psY[j][:])
                        if i == 0 and sidx in events0:
                            for fn in events0[sidx]:
                                fn()
                        if sidx % SPB == SPB - 1 or sidx == N_STEPS - 1:
                            nrows = 2 * (sidx % SPB + 1)
                            b0 = (sidx - sidx % SPB) * 2
                            blk = sidx // SPB
                            if i == 0 and blk < 3:
                                rings = [nc.sync, nc.scalar,
                                         nc.sync, nc.scalar]
                            else:
                                R3 = [nc.sync, nc.scalar, nc.gpsimd]
                                rings = [R3[(4 * blk + dd) % 3]
                                         for dd in range(4)]
                            srcs = (stageT[0:64, 0:nrows, :],
                                    stageT[64:128, 0:nrows, :],
                                    stageB[0:64, 0:nrows, :],
                                    stageB[64:128, 0:nrows, :])
                            for dd, (src, q) in enumerate(zip(srcs, rings)):
                                q.dma_start(
                                    y[i, :, 64 * dd + b0:64 * dd + b0 + nrows,
                                      :], src)
    nc.compile()
    return nc


_NC_CACHE = {}


def _get_nc(npx=2):
    if npx not in _NC_CACHE:
        _NC_CACHE[npx] = build_nc(npx)
    return _NC_CACHE[npx]


def _prep_shared(weight, fc_w, fc_b):
    # [E, O, I, KH, KW] -> [I, E, KH, KW, O] -> [64, E*9*64], dup halves
    wt = np.ascontiguousarray(weight.transpose(2, 0, 3, 4, 1)).reshape(
        C_IN, E * NTAP * C_OUT)
    wt = np.concatenate([wt, wt], axis=0).astype(np.float32)
    fcw = np.concatenate([fc_w.T, fc_w.T], axis=0).astype(np.float32)
    fcb = np.tile(fc_b.reshape(1, E), (128, 1)).astype(np.float32)
    ones = np.ones((128, 128), np.float32)
    return wt, fcw, fcb, ones


def kernel(inputs, weight, fc_w, fc_b, stride=1, dilation=1, padding=1,
           _trace=False, _npx=2):
    assert int(stride) == 1 and int(dilation) == 1 and int(padding) == 1
    inputs = np.asarray(inputs, dtype=np.float32)
    B = inputs.shape[0]
    assert B == N_CORES * IMGS_PER_CORE
    wt, fcw, fcb, ones = _prep_shared(np.asarray(weight), np.asarray(fc_w),
                                      np.asarray(fc_b))
    nc = _get_nc(_npx)
    in_maps = []
    for c in range(N_CORES):
        in_maps.append({
            "x": np.ascontiguousarray(inputs[2 * c:2 * c + 2]),
            "wt": wt, "fcw": fcw, "fcb": fcb, "ones": ones,
        })
    res = run_bass_kernel_spmd(nc, in_maps, core_ids=list(range(N_CORES)),
                               trace=_trace)
    out = np.concatenate([res.results[c]["y"] for c in range(N_CORES)], axis=0)
    if _trace:
        return out, res
    return out


# revision 18
# speedup vs baseline: 1.1274x; 1.0975x over previous
"""CondConv2D Trainium2 kernel (v5).

Problem (hardcoded): B=16, C_in=64, H=W=256, E=4, C_out=64, 3x3 conv,
stride=1, dilation=1, padding=1.

Sharding: data-parallel over batch. 8 cores x 2 images each. Expert weights
and routing fc params replicated (host pre-transposed for layout only).

Per-core kernel:
  - Each image resident in SBUF as ONE flat tile [128, 130, 256] bf16:
    partitions 0-63 hold channel c's TOP rows (slot m = global row m-1,
    incl. zero pad row -1), partitions 64-127 the BOTTOM rows
    (slot m = global row 127+m, incl. zero pad row 256). Loaded in 5 row
    chunks (SWDGE f32->bf16 cast DMAs). Flat layout => no row-pair ever
    straddles a tile boundary.
  - Routing runs on the PE so the drain engines stay free: fc-first
    pooling z[e,f] += fcw^T @ x accumulated over all row-pairs into one
    PSUM bank (both halves via quadrants (0,0)/(64,64), M=E columns),
    batched per load chunk; then DVE free-dim reduce -> PE transpose (into
    the same bank) -> DVE add halves + bias -> ACT sigmoid (partition 0)
    -> gpsimd.partition_broadcast -> DVE wmix = sum_e r_e W_e. Image 0's
    batches interleave with its loads (PE idle); image 1's are emitted
    between image 0's conv groups, paced to its chunk arrivals.
  - Conv: 32 steps/image; step i computes 4 output row-pairs via 4-quadrant
    PE packing: top pair i (0,0)->psX[0:64], top pair 32+i (0,64)->
    psX[64:128], bottom pair i (64,0)->psY[0:64], bottom pair 32+i
    (64,64)->psY[64:128]. Every tap is one matmul (N=512 center column,
    N=510 shifted columns). Tap-outer over groups of NB=3 steps keeps
    same-stationary matmuls back-to-back.
  - psX partitions hold (top q0 | top q1) rows at the same slot, psY
    (bottom q0 | bottom q1) => drains are partition-uniform copies into
    stage tiles [128, 12, 256] f32: X on ACT, Y on DVE.
  - Output: per 6-step block, 4 HWDGE DMAs with 12KB-contiguous
    per-partition descriptors, spread over the sync + scalar rings (plus
    the SWDGE ring once input loads are done) so multiple SDMA engine sets
    run concurrently.
"""
import sys

if "/opt/trn_rl_repo" not in sys.path:
    sys.path.insert(0, "/opt/trn_rl_repo")

import numpy as np

import concourse.bacc as bacc
import concourse.mybir as mybir
import concourse.tile as tile
from concourse.bass_utils import run_bass_kernel_spmd

F32 = mybir.dt.float32
BF16 = mybir.dt.bfloat16
AF = mybir.ActivationFunctionType
ALU = mybir.AluOpType
AX = mybir.AxisListType

N_CORES = 8
IMGS_PER_CORE = 2
C_IN = 64
C_OUT = 64
H = 256
W = 256
E = 4
NTAP = 9
SLOTS = 130           # 129 data rows + 1 pad row per half
CHUNK_BOUNDS = [0, 28, 56, 84, 112, 130]   # load chunk slot ranges
NCHUNK = len(CHUNK_BOUNDS) - 1
NB = 3                # conv steps per tap-outer group (2 psum banks/step)
SPB = 6               # conv steps per output stage block
STAGE_ROWS = 2 * SPB  # output rows per quarter per stage block
N_STEPS = 32          # steps per image (each step = 4 output row-pairs)
# center-column taps first (start=True must cover the full psum bank)
TAP_ORDER = [4, 1, 7, 3, 5, 0, 2, 6, 8]
# image-1 routing emission points in image 0's drain loop (step indices),
# matched to image 1's chunk DMA arrival times
Z1_BATCH_STEPS = [3, 7, 11, 15, 18]
Z1_REDUCE_STEP = 22
Z1_TAIL_STEP = 25     # transpose + bias/sigmoid/broadcast
Z1_WMIX_STEP = 26
NPX = 2  # legacy knob for test.py compatibility


def _z_pairs(k):
    """Odd slot indices m (matmul reads slots [m, m+1]) in load chunk k.

    Union over k = odd m in [1,127] => slots 1..128 each counted once per
    half (top rows 0..127, bottom rows 128..255), pad slots excluded."""
    lo, hi = CHUNK_BOUNDS[k], CHUNK_BOUNDS[k + 1]
    return [m for m in range(1, 128, 2) if lo <= m + 1 < hi]


def build_nc(npx=2):
    nc = bacc.Bacc("TRN2", target_bir_lowering=False, debug=False,
                   num_devices=N_CORES)
    x = nc.dram_tensor("x", [IMGS_PER_CORE, C_IN, H, W], F32,
                       kind="ExternalInput")
    wt = nc.dram_tensor("wt", [128, E * NTAP * C_OUT], F32,
                        kind="ExternalInput")
    fcw = nc.dram_tensor("fcw", [128, E], F32, kind="ExternalInput")
    fcb = nc.dram_tensor("fcb", [128, E], F32, kind="ExternalInput")
    ones = nc.dram_tensor("ones", [128, 128], F32, kind="ExternalInput")
    y = nc.dram_tensor("y", [IMGS_PER_CORE, C_OUT, H, W], F32,
                       kind="ExternalOutput")
    S = NTAP * C_OUT

    with tile.TileContext(nc) as tc:
        with (
            tc.tile_pool(name="consts", bufs=1) as consts,
            tc.tile_pool(name="img", bufs=IMGS_PER_CORE) as img_pool,
            tc.tile_pool(name="small", bufs=1) as small,
            tc.tile_pool(name="mix", bufs=1) as mix_pool,
            tc.tile_pool(name="stage", bufs=2) as stage_pool,
            tc.tile_pool(name="psum", bufs=7, space="PSUM") as psum_pool,
        ):
            wtt = consts.tile([128, E * S], BF16)
            fcwt = consts.tile([128, E], BF16)
            fcbt = consts.tile([128, E], F32)
            ident = consts.tile([128, 128], F32)
            nc.gpsimd.dma_start(fcwt[:], fcw[:])   # cast f32->bf16
            nc.sync.dma_start(fcbt[:], fcb[:])
            nc.sync.dma_start(ident[:], ones[:])

            # ---- per-image state ----
            xt = [None] * IMGS_PER_CORE
            zps = [None] * IMGS_PER_CORE
            wmix = [None] * IMGS_PER_CORE
            zr = [None] * IMGS_PER_CORE
            rt = [None] * IMGS_PER_CORE

            def emit_zbatch(i, k):
                """PE pooling batch for chunk k: z += fcw^T @ x row-pairs."""
                if k == 0:
                    zps[i] = psum_pool.tile([128, 2 * W], F32, name="z",
                                            bufs=1)
                z = zps[i]
                pairs = _z_pairs(k)
                last = k == NCHUNK - 1
                for idx, m in enumerate(pairs):
                    st = k == 0 and idx == 0
                    sp = last and idx == len(pairs) - 1
                    rt_ = xt[i][0:64, m:m + 2, :].rearrange("p a b -> p (a b)")
                    rb_ = xt[i][64:128, m:m + 2, :].rearrange(
                        "p a b -> p (a b)")
                    nc.tensor.matmul(z[0:E, :], fcwt[0:64, :], rt_,
                                     start=st, stop=False,
                                     tile_position=(0, 0),
                                     skip_group_check=True)
                    nc.tensor.matmul(z[64:64 + E, :], fcwt[64:128, :], rb_,
                                     start=st, stop=sp,
                                     tile_position=(64, 64),
                                     skip_group_check=True)

            def emit_zreduce(i):
                zr[i] = small.tile([128, 1], F32, name=f"zr{i}")
                nc.vector.reduce_sum(zr[i][:], zps[i][:], axis=AX.X)

            def emit_routing_tail(i):
                z = zps[i]
                # transpose zr into the (already consumed) z bank: row 0
                nc.tensor.transpose(z[0:1, 0:128], zr[i][:], ident[:])
                zrow = small.tile([1, 128], F32, name=f"zrow{i}")
                nc.vector.tensor_copy(zrow[:], z[0:1, 0:128])
                lsum = small.tile([1, E], F32, name=f"lsum{i}")
                nc.vector.tensor_tensor(lsum[:], zrow[0:1, 0:E],
                                        zrow[0:1, 64:64 + E], op=ALU.add)
                logits = small.tile([1, E], F32, name=f"logits{i}")
                nc.vector.tensor_scalar_mul(logits[:], lsum[:],
                                            1.0 / float(H * W))
                nc.vector.tensor_tensor(logits[:], logits[:], fcbt[0:1, :],
                                        op=ALU.add)
                rrow = small.tile([1, E], F32, name=f"rrow{i}")
                nc.scalar.activation(rrow[:], logits[:], AF.Sigmoid)
                rt[i] = small.tile([128, E], F32, name=f"rt{i}")
                nc.gpsimd.partition_broadcast(rt[i][:], rrow[:], channels=128)

            def emit_wmix(i, wtmp):
                wmix[i] = mix_pool.tile([128, S], BF16, name=f"wmix{i}")
                nc.vector.tensor_scalar_mul(wmix[i][:], wtt[:, 0:S],
                                            rt[i][:, 0:1])
                for e_ in range(1, E):
                    nc.vector.tensor_scalar_mul(wtmp[:],
                                                wtt[:, e_ * S:(e_ + 1) * S],
                                                rt[i][:, e_:e_ + 1])
                    nc.vector.tensor_tensor(wmix[i][:], wmix[i][:], wtmp[:],
                                            op=ALU.add)

            # ---- phase A: tiles, memsets, loads; image-0 z on idle PE ----
            for i in range(IMGS_PER_CORE):
                xt[i] = img_pool.tile([128, SLOTS, W], BF16, name="xt")
                nc.vector.memset(xt[i][0:64, 0:1, :], 0.0)
                nc.vector.memset(xt[i][64:128, SLOTS - 1:SLOTS, :], 0.0)

            for k in range(NCHUNK):
                s, e = CHUNK_BOUNDS[k], CHUNK_BOUNDS[k + 1]
                ts = max(s, 1)
                nc.gpsimd.dma_start(xt[0][0:64, ts:e, :],
                                    x[0, :, ts - 1:e - 1, :])
                be = min(e, SLOTS - 1)
                nc.gpsimd.dma_start(xt[0][64:128, s:be, :],
                                    x[0, :, 127 + s:127 + be, :])
                emit_zbatch(0, k)
            nc.gpsimd.dma_start(wtt[:], wt[:])
            for k in range(NCHUNK):
                s, e = CHUNK_BOUNDS[k], CHUNK_BOUNDS[k + 1]
                ts = max(s, 1)
                nc.gpsimd.dma_start(xt[1][0:64, ts:e, :],
                                    x[1, :, ts - 1:e - 1, :])
                be = min(e, SLOTS - 1)
                nc.gpsimd.dma_start(xt[1][64:128, s:be, :],
                                    x[1, :, 127 + s:127 + be, :])

            # ---- phase B: image 0 routing tail + wmix ----
            emit_zreduce(0)
            emit_routing_tail(0)
            wtmp0 = mix_pool.tile([128, S], BF16)
            emit_wmix(0, wtmp0)
            wtmp1 = mix_pool.tile([128, S], BF16)

            events0 = {}
            for bi, st_ in enumerate(Z1_BATCH_STEPS):
                events0.setdefault(st_, []).append(
                    lambda bi=bi: emit_zbatch(1, bi))
            events0.setdefault(Z1_REDUCE_STEP, []).append(
                lambda: emit_zreduce(1))
            events0.setdefault(Z1_TAIL_STEP, []).append(
                lambda: emit_routing_tail(1))
            events0.setdefault(Z1_WMIX_STEP, []).append(
                lambda: emit_wmix(1, wtmp1))

            # ---- phase C: conv per image ----
            for i in range(IMGS_PER_CORE):
                xti = xt[i]
                stageT = None
                stageB = None
                step = 0
                while step < N_STEPS:
                    nsteps = min(NB, N_STEPS - step)
                    steps = list(range(step, step + nsteps))
                    step += nsteps
                    wm = wmix[i]
                    psX = [psum_pool.tile([128, 2, W], F32, name="ps",
                                          tag="ps") for _ in steps]
                    psY = [psum_pool.tile([128, 2, W], F32, name="ps",
                                          tag="ps") for _ in steps]
                    for tap in TAP_ORDER:
                        kh, kw = divmod(tap, 3)
                        st = tap == TAP_ORDER[0]
                        sp = tap == TAP_ORDER[-1]
                        lT = wm[0:64, tap * 64:(tap + 1) * 64]
                        lB = wm[64:128, tap * 64:(tap + 1) * 64]
                        for j, sidx in enumerate(steps):
                            units = (
                                (psX[j], lT, 2 * sidx + kh, (0, 0),
                                 slice(0, 64), slice(0, 64)),
                                (psX[j], lT, 2 * (32 + sidx) + kh, (0, 64),
                                 slice(64, 128), slice(0, 64)),
                                (psY[j], lB, 2 * sidx + kh, (64, 0),
                                 slice(0, 64), slice(64, 128)),
                                (psY[j], lB, 2 * (32 + sidx) + kh, (64, 64),
                                 slice(64, 128), slice(64, 128)),
                            )
                            for ps, lhsT, m, tp, osl, hs in units:
                                if kw == 1:
                                    rhs = xti[hs, m:m + 2, :]
                                    rhs = rhs.rearrange("p a b -> p (a b)")
                                    out = ps[osl].rearrange("p a b -> p (a b)")
                                    nc.tensor.matmul(out, lhsT, rhs,
                                                     start=st, stop=sp,
                                                     tile_position=tp,
                                                     skip_group_check=True)
                                elif kw == 0:
                                    # out col w <- in col w-1
                                    nc.tensor.matmul(
                                        ps[osl, :, 1:W], lhsT,
                                        xti[hs, m:m + 2, 0:W - 1],
                                        start=st, stop=sp, tile_position=tp,
                                        skip_group_check=True)
                                else:
                                    nc.tensor.matmul(
                                        ps[osl, :, 0:W - 1], lhsT,
                                        xti[hs, m:m + 2, 1:W],
                                        start=st, stop=sp, tile_position=tp,
                                        skip_group_check=True)
                    # drains (X on ACT, Y on DVE) + deferred emissions
                    for j, sidx in enumerate(steps):
                        if sidx % SPB == 0:
                            stageT = stage_pool.tile([128, STAGE_ROWS, W],
                                                     F32)
                            stageB = stage_pool.tile([128, STAGE_ROWS, W],
                                                     F32)
                        r0 = (sidx % SPB) * 2
                        nc.scalar.copy(stageT[:, r0:r0 + 2, :], psX[j][:])
                        nc.vector.tensor_copy(stageB[:, r0:r0 + 2, :],
                                              psY[j][:])
                        if i == 0 and sidx in events0:
                            for fn in events0[sidx]:
                                fn()
                        if sidx % SPB == SPB - 1 or sidx == N_STEPS - 1:
                            nrows = 2 * (sidx % SPB + 1)
                            b0 = (sidx - sidx % SPB) * 2
                            blk = sidx // SPB
                            if i == 0 and blk < 3:
                                rings = [nc.sync, nc.scalar,
                                         nc.sync, nc.scalar]
                            else:
                                R3 = [nc.sync, nc.scalar, nc.gpsimd]
                                rings = [R3[(4 * blk + dd) % 3]
                                         for dd in range(4)]
                            srcs = (stageT[0:64, 0:nrows, :],
                                    stageT[64:128, 0:nrows, :],
                                    stageB[0:64, 0:nrows, :],
                                    stageB[64:128, 0:nrows, :])
                            for dd, (src, q) in enumerate(zip(srcs, rings)):
                                q.dma_start(
                                    y[i, :, 64 * dd + b0:64 * dd + b0 + nrows,
                                      :], src)
    nc.compile()
    return nc


_NC_CACHE = {}


def _get_nc(npx=2):
    if npx not in _NC_CACHE:
        _NC_CACHE[npx] = build_nc(npx)
    return _NC_CACHE[npx]


def _prep_shared(weight, fc_w, fc_b):
    # [E, O, I, KH, KW] -> [I, E, KH, KW, O] -> [64, E*9*64], dup halves
    wt = np.ascontiguousarray(weight.transpose(2, 0, 3, 4, 1)).reshape(
        C_IN, E * NTAP * C_OUT)
    wt = np.concatenate([wt, wt], axis=0).astype(np.float32)
    fcw = np.concatenate([fc_w.T, fc_w.T], axis=0).astype(np.float32)
    fcb = np.tile(fc_b.reshape(1, E), (128, 1)).astype(np.float32)
    ident = np.eye(128, dtype=np.float32)
    return wt, fcw, fcb, ident


def kernel(inputs, weight, fc_w, fc_b, stride=1, dilation=1, padding=1,
           _trace=False, _npx=2):
    assert int(stride) == 1 and int(dilation) == 1 and int(padding) == 1
    inputs = np.asarray(inputs, dtype=np.float32)
    B = inputs.shape[0]
    assert B == N_CORES * IMGS_PER_CORE
    wt, fcw, fcb, ident = _prep_shared(np.asarray(weight), np.asarray(fc_w),
                                       np.asarray(fc_b))
    nc = _get_nc(_npx)
    in_maps = []
    for c in range(N_CORES):
        in_maps.append({
            "x": np.ascontiguousarray(inputs[2 * c:2 * c + 2]),
            "wt": wt, "fcw": fcw, "fcb": fcb, "ones": ident,
        })
    res = run_bass_kernel_spmd(nc, in_maps, core_ids=list(range(N_CORES)),
                               trace=_trace)
    out = np.concatenate([res.results[c]["y"] for c in range(N_CORES)], axis=0)
    if _trace:
        return out, res
    return out
